# revision 27
# baseline (speedup 1.0000x reference)
"""BiAttention kernel for Trainium2 (Bass/Tile), 8-core data-parallel over batch.

Reference computation (per batch example):
    input_dot[l]  = input @ w_input                    [L]
    memory_dot[m] = memory @ w_memory                  [M]
    cross[l,m]    = (input * dot_scale) @ memory^T     [L,M]
    att = input_dot + memory_dot + cross
    att = where(mask_l | mask_m, -1e20, att)
    w1 = softmax_m(att); o1 = w1 @ memory
    w2 = softmax_l(max_m(att)); o2 = w2 @ input        [1,D]
    out = concat([input, o1, input*o1, o2*o1], -1)     [L,4D]

Sharding: batch 16 -> 2 examples per core across 8 cores; D-sized vectors
replicated. Each core runs an identical NEFF on its own slice.

Precision strategy (validated vs reference, rel err ~7e-3 < 2e-2):
  - score matmul in bf16 (memT/siT bf16), accumulated fp32 in PSUM
  - mask NEG constant chosen bf16-exact so masked-row exp(att-rowmax)=1
  - P = exp(att-rowmax) emitted as fp8e4m3 by the Act engine (accum=rowsum)
  - output_one matmul in fp8 with DoubleRow perf mode (2x PE rate)
  - o1 kept SBUF-resident in bf16 so block 4 (o2*o1) never re-reads HBM

Engine split: PE transposes+matmuls; DVE softmax stats + att2 affine;
Act dtype-converting copies + exp; Pool (gpsimd) elementwise block math.
"""

import sys

sys.path.insert(0, "/opt/trn_rl_repo")

import numpy as np
import ml_dtypes

import concourse.bass as bass
import concourse.tile as tile
from concourse import bacc, mybir
from concourse.bass import ds, ts
from concourse.bass_utils import run_bass_kernel_spmd

F32 = mybir.dt.float32
F32R = mybir.dt.float32r
BF16 = mybir.dt.bfloat16
F8 = mybir.dt.float8e4
U8 = mybir.dt.uint8
# bf16-exact big-negative: float32(bfloat16(-1e20)). Exactness matters: masked
# rows rely on att2 (bf16) == rowmax (f32) so exp(att2-rowmax) == 1.
NEG = -9.972771014849226e+19
P = 128
DR = mybir.MatmulPerfMode.DoubleRow


def _r(ap):
    return ap.bitcast(F32R)


def _f(ap):
    return ap.bitcast(F32)


def biattn_tile_kernel(tc, out_ap, inp_ap, mem_ap, msk_ap, w_in_ap, w_mem_ap,
                       dscale_ap, BPC, L, D, M):
    nc = tc.nc
    KD = D // P            # d-chunks (contraction tiles for score matmul)
    NLT = L // P           # l-tiles
    NMC = M // P           # m-chunks
    AC = 512               # att column chunk (PSUM bank, fp32)
    NAC = M // AC
    X = mybir.AxisListType.X
    Exp = mybir.ActivationFunctionType.Exp
    Copy = mybir.ActivationFunctionType.Copy
    Mult = mybir.AluOpType.mult
    Add = mybir.AluOpType.add

    ident_dram = nc.inline_tensor(np.eye(P, dtype=np.float32), name="identconst")
    ident8_dram = nc.inline_tensor(
        np.eye(P).astype(ml_dtypes.float8_e4m3), name="ident8const")
    onesb_dram = nc.inline_tensor(
        np.ones((1, P)).astype(ml_dtypes.bfloat16), name="onesbconst")

    import contextlib
    ctx = contextlib.ExitStack()
    with ctx:
        # --- pools ---
        consts = ctx.enter_context(tc.tile_pool(name="consts", bufs=1))
        residents = ctx.enter_context(tc.tile_pool(name="residents", bufs=1))
        mempool = ctx.enter_context(tc.tile_pool(name="mempool", bufs=3))
        inpool = ctx.enter_context(tc.tile_pool(name="inpool", bufs=3))
        sitpool = ctx.enter_context(tc.tile_pool(name="sitpool", bufs=2))
        att2pool = ctx.enter_context(tc.tile_pool(name="att2pool", bufs=2))
        p8pool = ctx.enter_context(tc.tile_pool(name="p8pool", bufs=2))
        ptsbpool = ctx.enter_context(tc.tile_pool(name="ptsbpool", bufs=2))
        outpool = ctx.enter_context(tc.tile_pool(name="outpool", bufs=2))
        b3pool = ctx.enter_context(tc.tile_pool(name="b3pool", bufs=2))
        smalls = ctx.enter_context(tc.tile_pool(name="smalls", bufs=3))
        attps = ctx.enter_context(tc.tile_pool(name="attps", bufs=3, space="PSUM"))
        tpps = ctx.enter_context(tc.tile_pool(name="tpps", bufs=1, space="PSUM"))
        pthps = ctx.enter_context(tc.tile_pool(name="pthps", bufs=1, space="PSUM"))
        o2ps = ctx.enter_context(tc.tile_pool(name="o2ps", bufs=1, space="PSUM"))
        drampool = ctx.enter_context(tc.tile_pool(name="drampool", bufs=1, space="DRAM"))

        # --- constants ---
        ident_r = consts.tile([P, P], F32R)     # for f32r-mode PE transposes
        nc.sync.dma_start(out=ident_r, in_=_r(ident_dram.ap()))
        ident_8 = consts.tile([P, P], F8)       # for fp8-mode PE transposes
        nc.sync.dma_start(out=ident_8, in_=ident8_dram.ap())
        ones_b = consts.tile([1, P], BF16)      # K=1 stationary for extra row
        nc.sync.dma_start(out=ones_b, in_=onesb_dram.ap())
        ones_f = consts.tile([P, 1], F32)       # fp32 reduction helper
        nc.vector.memset(ones_f, 1.0)
        neg30 = consts.tile([P, 1], F32)        # bias for shifted exp (o2 path)
        nc.vector.memset(neg30, -30.0)
        w_mem_f = consts.tile([P, KD], F32)     # w_memory d-major staging
        ds_t = consts.tile([P, KD], F32)        # dot_scale in d-major layout
        for k in range(KD):
            nc.sync.dma_start(out=w_mem_f[:, k:k + 1],
                              in_=w_mem_ap[ts(k, P)].unsqueeze(-1))
            nc.sync.dma_start(out=ds_t[:, k:k + 1],
                              in_=dscale_ap[ts(k, P)].unsqueeze(-1))
        w_mem_t = consts.tile([P, KD], BF16)    # bf16 for fp32-free matmul
        nc.vector.tensor_copy(out=w_mem_t, in_=w_mem_f)
        w_in_bcast = consts.tile([P, D], F32)   # w_input replicated on partitions
        nc.sync.dma_start(
            out=w_in_bcast,
            in_=bass.AP(tensor=w_in_ap.tensor, offset=w_in_ap.offset,
                        ap=[[0, P]] + list(w_in_ap.ap)),
        )

        def emit_phaseB(b0, o1b0, o2b0):
            # out[...,3D:4D] = o2 * o1 from resident bf16 o1; deferred into
            # the next example's phase 0/A so its DMA writes overlap compute.
            for lt in range(NLT):
                b3 = b3pool.tile([P, D], F32, tag="b3")
                nc.vector.tensor_tensor(out=b3, in0=o1b0[:, lt, :], in1=o2b0,
                                        op=Mult)
                nc.sync.dma_start(out=out_ap[b0, ts(lt, P), 3 * D:4 * D],
                                  in_=b3)

        pendingB = None
        for b in range(BPC):
            # === phase 0: memory resident structures ===
            mask_row = residents.tile([1, M], U8, tag="mrow")
            nc.sync.dma_start(out=mask_row, in_=msk_ap[b:b + 1, :])

            memT = residents.tile([P, KD, M], BF16, tag="memT")
            mem8 = residents.tile([P, NMC, D], F8, tag="mem8")
            for mc in range(NMC):
                mchunk = mempool.tile([P, D], F32R, tag="mchunk")
                nc.sync.dma_start(out=mchunk, in_=_r(mem_ap[b, ts(mc, P), :]))
                nc.vector.tensor_copy(out=mem8[:, mc, :], in_=_f(mchunk))
                tp8 = tpps.tile([P, KD, P], F32R, tag="tps")
                for k in range(KD):
                    nc.tensor.transpose(tp8[:, k, :], mchunk[:, ts(k, P)],
                                        ident_r)
                nc.scalar.activation(out=memT[:, :, ds(mc * P, P)],
                                     in_=_f(tp8), func=Copy)

            # extra_row[m] = memory_dot[m] + NEG*mask_m   (bf16)
            extra_row = residents.tile([1, M], BF16, tag="erow")
            for g in range(NAC):
                mdot_ps = attps.tile([1, AC], F32, tag="att")
                for k in range(KD):
                    nc.tensor.matmul(mdot_ps, w_mem_t[:, k:k + 1],
                                     memT[:, k, ds(g * AC, AC)],
                                     start=(k == 0), stop=(k == KD - 1))
                mneg = smalls.tile([1, AC], F32, tag="mneg")
                nc.vector.tensor_scalar(out=mneg,
                                        in0=mask_row[0:1, ds(g * AC, AC)],
                                        scalar1=NEG, scalar2=0.0,
                                        op0=Mult, op1=Add)
                nc.vector.tensor_add(out=extra_row[0:1, ds(g * AC, AC)],
                                     in0=mneg, in1=mdot_ps)

            # deferred previous-example phase B: issued after this example's
            # phase 0 so its DMA writes ride behind the memory prefetch.
            if pendingB is not None:
                emit_phaseB(*pendingB)
                pendingB = None

            # === phase A: per l-tile scores/softmax/o1, software-pipelined ===
            # o1/block emission for tile lt-1 is interleaved into tile lt so
            # the PE fills the softmax (DVE/Act) latency with o1 matmuls.
            nrm_all = residents.tile([P, NLT], F32, tag="nrmall")
            o1b = residents.tile([P, NLT, D], BF16, tag="o1b", bufs=2)
            o2_ps = o2ps.tile([1, D], F32, tag="o2")

            def emit_o1_matmuls(st):
                # output_one = P @ memory, fp8 DoubleRow (2x rate); PE only
                st["o1slots"] = []
                for half in range(2):
                    o1slot = attps.tile([P, AC], F32, tag="att")
                    for dg in range(2):
                        osl = ds(dg * 256, 256)
                        msl = ds(half * 512 + dg * 256, 256)
                        for j in range(NMC // 2):
                            nc.tensor.matmul(
                                o1slot[:, osl],
                                st["PT"][:, 2 * j:2 * j + 2, :],
                                mem8[:, 2 * j:2 * j + 2, msl],
                                start=(j == 0), stop=(j == NMC // 2 - 1),
                                perf_mode=DR)
                    st["o1slots"].append(o1slot)

            def emit_o1_drain(st):
                lt0 = st["lt"]
                lsl0 = ts(lt0, P)
                for half in range(2):
                    nc.scalar.activation(out=o1b[:, lt0, ds(half * 512, 512)],
                                         in_=st["o1slots"][half], func=Copy,
                                         scale=st["recip"])
                # blocks 0..2: out[...,0:D]=input, [D:2D]=o1, [2D:3D]=in*o1
                outbuf = outpool.tile([P, 2 * D], F32, tag="ob")
                nc.vector.tensor_copy(out=outbuf[:, 0:D], in_=o1b[:, lt0, :])
                nc.gpsimd.tensor_tensor(out=outbuf[:, D:2 * D],
                                        in0=_f(st["in_t"]),
                                        in1=o1b[:, lt0, :], op=Mult)
                nc.sync.dma_start(out=out_ap[b, lsl0, 0:D], in_=_f(st["in_t"]))
                nc.sync.dma_start(out=out_ap[b, lsl0, D:3 * D], in_=outbuf)

            prev = None
            for lt in range(NLT):
                lsl = ts(lt, P)
                in_t = inpool.tile([P, D], F32R, tag="in")
                nc.sync.dma_start(out=in_t, in_=_r(inp_ap[b, lsl, :]))
                mlt = smalls.tile([P, 1], U8, tag="mlt")
                nc.sync.dma_start(out=mlt, in_=msk_ap[b, lsl].unsqueeze(-1))

                # input_dot: Pool mul into att2's buffer as scratch (the
                # scratch is overwritten by scores later), DVE row-reduce
                att2 = att2pool.tile([P, M], BF16, tag="att2")
                idot = smalls.tile([P, 1], F32, tag="idot")
                nc.gpsimd.tensor_tensor(out=att2.bitcast(F32), in0=_f(in_t),
                                        in1=w_in_bcast, op=Mult)
                nc.vector.reduce_sum(out=idot, in_=att2.bitcast(F32), axis=X)

                # s1 = 1-mask_l ; s2 = idot*(1-mask_l) + NEG*mask_l
                mask_f = smalls.tile([P, 1], F32, tag="maskf")
                nc.vector.tensor_copy(out=mask_f, in_=mlt)
                s1 = smalls.tile([P, 1], F32, tag="s1")
                nc.vector.tensor_scalar(out=s1, in0=mask_f, scalar1=-1.0,
                                        scalar2=1.0, op0=Mult, op1=Add)
                s2t = smalls.tile([P, 1], F32, tag="s2t")
                nc.vector.tensor_tensor(out=s2t, in0=idot, in1=s1, op=Mult)
                s2m = smalls.tile([P, 1], F32, tag="s2m")
                nc.vector.tensor_scalar(out=s2m, in0=mask_f, scalar1=NEG,
                                        scalar2=0.0, op0=Mult, op1=Add)
                s2 = smalls.tile([P, 1], F32, tag="s2")
                nc.vector.tensor_add(out=s2, in0=s2m, in1=s2t)

                # scaled-input transpose: siT[d,k,l] = (input^T * dot_scale) bf16
                tp8 = tpps.tile([P, KD, P], F32R, tag="tps")
                for k in range(KD):
                    nc.tensor.transpose(tp8[:, k, :], in_t[:, ts(k, P)],
                                        ident_r)
                siT = sitpool.tile([P, KD, P], BF16, tag="sit")
                for k in range(KD):
                    nc.scalar.activation(out=siT[:, k, :], in_=_f(tp8[:, k, :]),
                                         func=Copy, scale=ds_t[:, k:k + 1])

                # scores: rank-1 extra row + bf16 cross, drained per group
                for g in range(NAC):
                    att_ps = attps.tile([P, AC], F32, tag="att")
                    nc.tensor.matmul(att_ps, ones_b,
                                     extra_row[0:1, ds(g * AC, AC)],
                                     start=True, stop=False)
                    for k in range(KD):
                        nc.tensor.matmul(att_ps, siT[:, k, :],
                                         memT[:, k, ds(g * AC, AC)],
                                         start=False, stop=(k == KD - 1))
                    nc.vector.tensor_scalar(
                        out=att2[:, ds(g * AC, AC)], in0=att_ps, scalar1=s1,
                        scalar2=s2, op0=Mult, op1=Add)
                # rowmax over the bf16 scores (exp(att2-rowmax) <= 1 exactly)
                rowmax = smalls.tile([P, 1], F32, tag="rowmax")
                nc.vector.reduce_max(out=rowmax, in_=att2, axis=X)
                nrm = nrm_all[:, lt:lt + 1]                 # -rowmax
                nc.vector.tensor_scalar(out=nrm, in0=rowmax, scalar1=-1.0,
                                        scalar2=0.0, op0=Mult, op1=Add)

                # output_two partials: ev = exp(rowmax - 30) (f32r for matmul)
                ev = smalls.tile([P, 1], F32R, tag="ev")
                nc.scalar.activation(out=ev, in_=nrm, func=Exp, scale=-1.0,
                                     bias=neg30)
                for h in range(2):
                    nc.tensor.matmul(o2_ps[0:1, ds(h * 512, 512)], ev,
                                     in_t[:, ds(h * 512, 512)],
                                     start=(lt == 0), stop=(lt == NLT - 1))

                # deferred o1 matmuls for the previous tile: fill the PE
                # bubble while DVE/Act finish this tile's softmax stats.
                if prev is not None:
                    emit_o1_matmuls(prev)

                # P = exp(att2 - rowmax) as fp8 + PT transposes, in M-halves
                # so Act(exp), PE(transpose) and DVE(compact) overlap.
                # HW fp8 transpose writes PSUM with element step 2, so the
                # out AP is stride-2 over a 2x-wide tile; DVE re-compacts.
                p8 = p8pool.tile([P, M], F8, tag="p8")
                rowsums = smalls.tile([P, 2], F32, tag="rsums")
                PT = ptsbpool.tile([P, NMC, P], F8, tag="pt")
                NMH = NMC // 2
                for h in range(2):
                    nc.scalar.activation(out=p8[:, ds(h * (M // 2), M // 2)],
                                         in_=att2[:, ds(h * (M // 2), M // 2)],
                                         func=Exp, bias=nrm, scale=1.0,
                                         accum_out=rowsums[:, h:h + 1])
                    ptp = pthps.tile([P, NMH, 2 * P], F8, tag="ptp")
                    for j in range(NMH):
                        sub = ptp[:, j, :]
                        out2 = bass.AP(tensor=sub.tensor, offset=sub.offset,
                                       ap=[list(sub.ap)[0], [2, P]])
                        nc.tensor.transpose(out2, p8[:, ts(h * NMH + j, P)],
                                            ident_8)
                    ptp_strided = bass.AP(
                        tensor=ptp.tensor, offset=ptp.offset,
                        ap=[list(ptp.ap)[0], [2 * P, NMH], [2, P]])
                    if h == 0:
                        nc.vector.tensor_copy(out=PT[:, ds(h * NMH, NMH), :],
                                              in_=ptp_strided)
                    else:
                        nc.scalar.activation(out=PT[:, ds(h * NMH, NMH), :],
                                             in_=ptp_strided, func=Copy)
                rowsum = smalls.tile([P, 1], F32, tag="rsum")
                nc.vector.tensor_add(out=rowsum, in0=rowsums[:, 0:1],
                                     in1=rowsums[:, 1:2])
                recip = smalls.tile([P, 1], F32, tag="recip")
                nc.vector.reciprocal(recip, rowsum)

                if prev is not None:
                    emit_o1_drain(prev)
                prev = {"lt": lt, "PT": PT, "recip": recip, "in_t": in_t}
            emit_o1_matmuls(prev)
            emit_o1_drain(prev)

            # === finalize output_two: o2 = o2_raw / sum(exp(rowmax-30)) ===
            evall = smalls.tile([P, NLT], F32, tag="evall")
            nc.scalar.activation(out=evall, in_=nrm_all, func=Exp, scale=-1.0,
                                 bias=neg30)
            colsum_ps = attps.tile([NLT, 1], F32, tag="att")
            nc.tensor.matmul(colsum_ps, evall, ones_f, start=True, stop=True)
            cs_sb = smalls.tile([NLT, 1], F32, tag="cssb")
            nc.vector.tensor_copy(out=cs_sb, in_=colsum_ps)
            z2_ps = attps.tile([1, 1], F32, tag="att")
            nc.tensor.matmul(z2_ps, cs_sb, ones_f[0:NLT, 0:1], start=True,
                             stop=True)
            z2r = smalls.tile([1, 1], F32, tag="z2r")
            nc.vector.reciprocal(z2r, z2_ps)
            o2b = residents.tile([P, D], F32, tag="o2b", bufs=2)
            nc.scalar.activation(out=o2b[0:1, :], in_=o2_ps[0:1, 0:D],
                                 func=Copy, bias=0.0, scale=z2r)
            o2_dram = drampool.tile([1, D], F32, tag="o2d")
            nc.sync.dma_start(out=o2_dram, in_=o2b[0:1, :])
            nc.sync.dma_start(
                out=o2b,
                in_=bass.AP(tensor=o2_dram.tensor, offset=o2_dram.offset,
                            ap=[[0, P]] + list(o2_dram.ap)[1:]))
            pendingB = (b, o1b, o2b)
        emit_phaseB(*pendingB)


def build_module(BPC, L, D, M, enable_asserts=False):
    nc = bacc.Bacc("TRN2", target_bir_lowering=False, debug=False,
                   enable_asserts=enable_asserts, num_devices=1)
    inp = nc.dram_tensor("input", (BPC, L, D), F32, kind="ExternalInput").ap()
    mem = nc.dram_tensor("memory", (BPC, M, D), F32, kind="ExternalInput").ap()
    msk = nc.dram_tensor("mask", (BPC, L), U8, kind="ExternalInput").ap()
    w_in = nc.dram_tensor("w_input", (D,), F32, kind="ExternalInput").ap()
    w_mem = nc.dram_tensor("w_memory", (D,), F32, kind="ExternalInput").ap()
    dsc = nc.dram_tensor("dot_scale", (D,), F32, kind="ExternalInput").ap()
    out = nc.dram_tensor("out", (BPC, L, 4 * D), F32, kind="ExternalOutput").ap()
    with tile.TileContext(nc) as tc:
        biattn_tile_kernel(tc, out, inp, mem, msk, w_in, w_mem, dsc,
                           BPC, L, D, M)
    nc.compile()
    return nc


_NC_CACHE = {}


def kernel(input, memory, mask, w_input, w_memory, dot_scale, trace=False):
    B, L, D = input.shape
    M = memory.shape[1]
    NCORES = 8
    BPC = B // NCORES
    key = (BPC, L, D, M)
    if key not in _NC_CACHE:
        _NC_CACHE[key] = build_module(*key)
    nc = _NC_CACHE[key]

    input = np.ascontiguousarray(np.asarray(input, dtype=np.float32))
    memory = np.ascontiguousarray(np.asarray(memory, dtype=np.float32))
    mask_u8 = np.ascontiguousarray(np.asarray(mask).astype(np.uint8))
    w_input = np.ascontiguousarray(np.asarray(w_input, dtype=np.float32))
    w_memory = np.ascontiguousarray(np.asarray(w_memory, dtype=np.float32))
    dot_scale = np.ascontiguousarray(np.asarray(dot_scale, dtype=np.float32))

    in_maps = []
    for c in range(NCORES):
        sl = slice(c * BPC, (c + 1) * BPC)
        in_maps.append({
            "input": input[sl], "memory": memory[sl], "mask": mask_u8[sl],
            "w_input": w_input, "w_memory": w_memory, "dot_scale": dot_scale,
        })
    res = run_bass_kernel_spmd(nc, in_maps, core_ids=list(range(NCORES)),
                               trace=trace)
    out = np.concatenate([res.results[c]["out"] for c in range(NCORES)], axis=0)
    if trace:
        kernel.last_exec_time_ns = res.exec_time_ns
        kernel.last_results = res
    return out
